# revision 114
# baseline (speedup 1.0000x reference)
"""Trainium2 Bass kernel: AttentionBlock (GroupNorm + self/cross QKV attention + proj + residual).

Data-parallel over batch: B=8, one batch element per NeuronCore (8 cores), no collectives.

v3 design (per core; C=768, T=1024, S=256, 12 heads x 64 ch). The v2 kernel was
DVE/ACT-bound (softmax exp + normalize ~120/114us busy). v3 restructures the
softmax consumer side so DVE/ACT do (almost) nothing but the irreducible exp:

  - Scores: fp8 DoubleRow, kd stationary / qd moving; BOTH heads of a step
    land in one [128, 1024] psum tile so each step is ONE exp op
    (amortizes the per-op PSUM/SBUF access latency).
  - exp -> fp8 E directly: ACT steps via ACT.Exp with fp8e4 output (~1038ns);
    DVE steps via an 8-bit Schraudolph (mult+add into int8 bits = fp8e4m3
    bit pattern, ~7% max elem err; the diffuse softmax averages it down),
    ~1192ns. E stored [128, 2(st parity), 2(head), 512].
  - PV flipped: E is the *stationary* operand (LdWeights is free in HW and
    cost model), vt the moving one: out a^T[t-chunk 128, 64] per (head,
    chunk), 33 cyc per fp8-DR matmul -> PV drops from 123k to ~16k PE
    cycles. The softmax denominator comes from 1-column matmuls against a
    constant fp8 ones vector into a [128, 16] psum tile (Z per (t, head)
    lands in a *column* = per-partition scalar for the normalize).
  - Normalize collapses to: one [128,8] reciprocal + one [128,512]
    stride-0-broadcast tensor_tensor multiply per half-pair, writing a^T in
    bf16 (vs v2's reciprocal-1024 / gpsimd broadcast / 3 multiplies).
  - a^T -> a via DMA xbar transpose (dma_start_transpose, 14ns/tile on the
    idle DMA engines), giving bf16 a in [c, t] layout; proj runs bf16
    (non-DR) which also removes v2's fp8-a quantization error. The LAST
    half uses PE transposes instead (the DMA route's ~3.5us latency would
    idle the PE into a p-state drop right before the tail proj).
  - x lives on-device only as host-cast bf16 (halves the x bytes on the
    serial DMA pipe that paces phase A); groupnorm stats via bn_stats; the
    residual is added on the PE via an identity matmul so the proj drain is
    a plain assignable copy.
  - Loop is t-half-MAJOR (all pairs at th=0, then th=1) so the th=0 halves
    of proj (6 j-chains + idn-residual per out-tile) run inside the loop's
    PE slack; only th=1 proj remains in the tail. q/k for pair j+2 are
    produced during pair j's th0 half (two-half lead hides the ~3us
    drain + shuffle-DMA + sem latency); pairs 0-1 come from phase A.
  - PSUM: scores 3x[128,1024] (productions and th0-proj ride the same
    rotation) + pa 1x[128,512] (a^T accum, 64-col groups) + pz 1 bank = 8.
  - Engine split of the 120 exp steps is Bresenham-interleaved (K_EXPDVE,
    default 0.50 = strict alternation); gn chain stays on DVE end-to-end
    (spreading it over Pool creates a Pool.SEQ head-of-line stall that
    blocks shuffle legs). Input DMAs are queue-placed by need time against
    the serial DMA pipe: wek/wq/bqc/wk/bkc ride the ACT queue behind its
    two x tiles so the first PE work (enc keys) isn't gated by the big
    weight transfers racing ahead on other queues.

TimelineSim: 138.0us (v2 baseline: 168.0us). HW rel err ~6e-3 (gate 2e-2).
"""

import os
import numpy as np
import ml_dtypes
from contextlib import ExitStack

from contextlib import nullcontext as _nullcm
import concourse.tile as tile
from concourse import bacc, mybir
from concourse.bass_utils import run_bass_kernel_spmd

F32 = mybir.dt.float32
BF16 = mybir.dt.bfloat16
FP8 = mybir.dt.float8e4
I8 = mybir.dt.int8
NPBF = ml_dtypes.bfloat16
NPF8 = ml_dtypes.float8_e4m3

B, C, HH, WW = 8, 768, 32, 32
T = HH * WW          # 1024
S = 256
EC = 768
NH, CH = 12, 64      # heads, head channels
NG = 32              # groupnorm groups
EPS = 1e-5
NP = C // 128        # 6 channel-partition tiles
NPAIR = NH // 2      # 6 head pairs
ST = S + T           # 1280 attention keys
NS = ST // 128       # 10 s-tiles
NSP = NS // 2        # 5 st-pair tiles (DR contraction pairs)
NK = 3               # ktile-pairs for C=768 contraction (3 x (2x128))
NCH = 4              # t-chunks per 512 t-half
SCALE = 1.0 / np.sqrt(np.sqrt(CH))

WS = 32.0            # fp8 weight scale for q/k/ek
WSV = 64.0           # fp8 weight scale for v/ev

AOP = mybir.AluOpType
ACT = mybir.ActivationFunctionType
DR = mybir.MatmulPerfMode.DoubleRow

# 8-bit Schraudolph: fp8e4m3 bits = trunc(EXP8_A * logit + EXP8_B); logits
# are ~N(0, 0.31) here (|logit| < ~2.1 measured), safe window is (-4.8, 6.1).
EXP8_A = 8.0 * float(np.log2(np.e))
EXP8_B = 56.125

# exp engine split: fraction of the 120 merged [128,1024] exp tiles on DVE
# (Schraudolph-8 int8 bits); rest on ACT (true Exp->fp8e4). DVE tile ~1192ns,
# ACT ~1038ns; DVE carries the divides/residuals so ACT takes the majority.
EXP_DVE_FRAC = float(os.environ.get("K_EXPDVE", "0.50"))


def _emit(tc, ins, out_ap):
    nc = tc.nc
    ctx = tc._ctx

    # ---------------- pools ----------------
    const = ctx.enter_context(tc.tile_pool(name="const", bufs=1))
    xpool = ctx.enter_context(tc.tile_pool(name="x", bufs=1))
    attn = ctx.enter_context(tc.tile_pool(name="attn", bufs=1))
    spool = ctx.enter_context(tc.tile_pool(name="small", bufs=4))
    opool = ctx.enter_context(tc.tile_pool(name="o", bufs=int(__import__("os").environ.get("K_OBUFS", "3"))))
    epool = ctx.enter_context(tc.tile_pool(name="E", bufs=int(__import__("os").environ.get("K_EBUFS", "10"))))
    zpool = ctx.enter_context(tc.tile_pool(name="z", bufs=int(__import__("os").environ.get("K_ZBUFS", "3"))))
    early = tc.alloc_tile_pool(name="early", bufs=1)
    sqpool = tc.alloc_tile_pool(name="sq", bufs=2)

    # ---------------- SBUF residents ----------------
    # x lives on-device ONLY as bf16 (host-cast): gn stats read it, xn reads
    # it, and the residual is added on the PE via an identity matmul (so the
    # proj drain is a plain copy). Halves the x DMA bytes on the serial DMA
    # pipe that paces phase A.
    x_bf = [xpool.tile([128, T], BF16, tag=f"xb{i}", name=f"xb_{i}")
            for i in range(NP)]
    xn_db = [xpool.tile([128, 2 * T], FP8, tag=f"xn{i}", name=f"xn_{i}")
             for i in range(NK)]
    q_f8 = [attn.tile([128, T], FP8, tag=f"q{j}", name=f"qf_{j}")
            for j in range(NPAIR)]
    k_f8 = [attn.tile([128, ST], FP8, tag=f"k{j}", name=f"kf_{j}")
            for j in range(NPAIR)]
    # DoubleRow-shuffled q/k: [32, head, ktile, t] (ch = 32*ktile + p)
    q_dr = [attn.tile([32, 2 * 2 * T], FP8, tag=f"qd{j}", name=f"qd_{j}")
            for j in range(NPAIR)]
    k_dr = [attn.tile([32, 2 * 2 * ST], FP8, tag=f"kd{j}", name=f"kd_{j}")
            for j in range(NPAIR)]
    # v^T per st-PAIR, DR rhs layout: [p, head, st-parity, ch] fp8 (plain v)
    vt_dr = [attn.tile([128, NH * 2 * CH], FP8, tag=f"vt{sp}", name=f"vt_{sp}")
             for sp in range(NSP)]
    # full a (post-softmax, normalized) in [c, t] layout, bf16, per pair
    a_sb = [attn.tile([128, T], BF16, tag=f"a{j}", name=f"a_{j}")
            for j in range(NPAIR)]

    wq_sb = const.tile([128, NPAIR * NK * 2 * 128], FP8, tag="wq")
    wk_sb = const.tile([128, NPAIR * NK * 2 * 128], FP8, tag="wk")
    wv_sb = const.tile([128, NK * 2 * C], FP8, tag="wv")
    wek_sb = early.tile([128, NPAIR * NK * 2 * 128], FP8, tag="wek")
    wev_sb = early.tile([128, NK * 2 * C], FP8, tag="wev")
    # proj weights bf16, [c-part, j, ot, m] (lhsT per (j, ot))
    wpt_sb = const.tile([128, NPAIR * NP * 128], BF16, tag="wpt")
    enc_sb = early.tile([128, NK * 2 * S], FP8, tag="enc")

    bev_sb = early.tile([1, C], BF16, tag="bev")
    bqc_sb = const.tile([128, NPAIR], F32, tag="bqc")
    bkc_sb = const.tile([128, NPAIR], F32, tag="bkc")
    bekc_sb = const.tile([128, NPAIR], F32, tag="bekc")

    gnw_sb = const.tile([128, NP], F32, tag="gnw")
    gnb_sb = const.tile([128, NP], F32, tag="gnb")
    ind_sb = early.tile([128, NP * NG], F32, tag="ind")
    indT_sb = early.tile([32, C], F32, tag="indT")

    ones_r = early.tile([1, 128], BF16, tag="ones_r")
    ones_f8 = const.tile([128, 2], FP8, tag="ones_f8")
    zeros_c = const.tile([128, 1], F32, tag="zeros_c")
    idn_sb = const.tile([128, 128], BF16, tag="idn")

    s12_sb = const.tile([128, 2 * NP], F32, tag="s12")
    ab_sb = const.tile([128, 2 * NP], F32, tag="ab")

    # ---------------- input DMAs ----------------
    nc.vector.memset(ones_r[:], 1.0)
    nc.vector.memset(ones_f8[:], 1.0)
    nc.vector.memset(zeros_c[:], 0.0)
    warm_t = const.tile([1, 1], F32, tag="warm")
    nc.scalar.activation(warm_t[:], zeros_c[0:1, 0:1], ACT.Exp)

    # input DMAs ordered by NEED time -- the model's DMA_ENGINES resource is
    # serial, so transfers land roughly in issue order. x tiles gate the
    # stats chain; enc/wek gate the first PE work; wpt isn't needed till
    # the proj chains (~60us in).
    for ct in range(NP):
        eng = (nc.sync, nc.scalar, nc.gpsimd)[ct % 3]
        eng.dma_start(x_bf[ct][:], ins["xbf"][128 * ct: 128 * (ct + 1), :])
    for nm, dst in (("enc", enc_sb), ("ind", ind_sb), ("indT", indT_sb),
                    ("gnw", gnw_sb), ("gnb", gnb_sb)):
        nc.sync.dma_start(dst[:], ins[nm])
    for nm, dst in (("wek", wek_sb), ("wq", wq_sb), ("bqc", bqc_sb),
                    ("wk", wk_sb), ("bkc", bkc_sb)):
        nc.scalar.dma_start(dst[:], ins[nm])
    for nm, dst in (("wv", wv_sb), ("wev", wev_sb), ("bev", bev_sb),
                    ("bekc", bekc_sb), ("wpt", wpt_sb), ("idn", idn_sb)):
        nc.gpsimd.dma_start(dst[:], ins[nm])

    # DR-layout views
    def w_pair_view(w, j):
        v = w[:].rearrange("p (j k i m) -> p j k i m", j=NPAIR, k=NK, i=2)
        return v[:, j]

    wv_v = wv_sb[:].rearrange("p (k i n) -> p k i n", k=NK, i=2)
    wev_v = wev_sb[:].rearrange("p (k i n) -> p k i n", k=NK, i=2)
    wpt_v = wpt_sb[:].rearrange("p (j o m) -> p j o m", j=NPAIR, o=NP)
    enc_v = enc_sb[:].rearrange("p (k i n) -> p k i n", k=NK, i=2)
    xn_v = [t[:].rearrange("p (i n) -> p i n", i=2) for t in xn_db]
    vt4 = [t[:].rearrange("p (h i c) -> p h i c", h=NH, i=2) for t in vt_dr]
    qd_v = [t[:].rearrange("p (hh ii n) -> p hh ii n", hh=2, ii=2) for t in q_dr]
    kd_v = [t[:].rearrange("p (hh ii n) -> p hh ii n", hh=2, ii=2) for t in k_dr]

    def emit_shuffle(j, which):
        # partition-offset block DMAs: rows 64*hh+32*ii -> partition-0 block;
        # alternate HWDGE (sync) / SWDGE (gpsimd) so the 4 copies overlap
        src_t, dst_t, w = (
            (q_f8[j], q_dr[j], T) if which == "q"
            else (k_f8[j], k_dr[j], ST))
        for hh in range(2):
            for ii in range(2):
                r = 64 * hh + 32 * ii
                eng = nc.sync if (ii == (which == "q")) else nc.gpsimd
                eng.dma_start(
                    dst_t[0:32, (2 * hh + ii) * w: (2 * hh + ii + 1) * w],
                    src_t[r: r + 32, :],
                )

    # full q or k production for one pair into a [128, T] psum tile;
    # drain engine alternates ACT/DVE for load balance
    def emit_prod(j, which, pq, eng=None):
        w_sb, bc_sb, dst_base = (
            (wq_sb, bqc_sb, q_f8[j][:, 0:]) if which == "q"
            else (wk_sb, bkc_sb, k_f8[j][:, S:])
        )
        wj = w_pair_view(w_sb, j)
        for c in range(4):
            for kp in range(NK):
                nc.tensor.matmul(
                    pq[:, 256 * c: 256 * (c + 1)],
                    lhsT=wj[:, kp],
                    rhs=xn_v[kp][:, :, 256 * c: 256 * (c + 1)],
                    start=(kp == 0 and c % 2 == 0), stop=(kp == NK - 1),
                    perf_mode=DR, skip_group_check=True,
                )
        if eng is None or eng is nc.scalar:
            nc.scalar.activation(
                dst_base[:, 0: T], pq[:, 0:T], ACT.Identity,
                bias=bc_sb[:, j: j + 1], scale=1.0,
            )
        else:
            nc.vector.tensor_scalar(
                dst_base[:, 0: T], pq[:, 0:T], 1.0,
                bc_sb[:, j: j + 1], op0=AOP.mult, op1=AOP.add,
            )
        emit_shuffle(j, which)

    # v^T production for one self t-tile (st = 2 + tt) into [128, C] psum;
    # drain alternates DVE/ACT
    def emit_vt_tt(tt, pvt):
        st = 2 + tt
        for cs in range(NK):
            for kp in range(NK):
                nc.tensor.matmul(
                    pvt[:, 256 * cs: 256 * (cs + 1)],
                    lhsT=xn_v[kp][:, :, 128 * tt: 128 * (tt + 1)],
                    rhs=wv_v[:, kp, :, 256 * cs: 256 * (cs + 1)],
                    start=(kp == 0 and cs != 1), stop=(kp == NK - 1),
                    perf_mode=DR, skip_group_check=True,
                )
        dst = vt4[st // 2][:, :, st % 2, :]
        src = pvt[:, 0:C].rearrange("p (h c) -> p h c", c=CH)
        if tt % 2 == 0:
            nc.vector.tensor_scalar(dst, src, 1.0 / WSV, None, op0=AOP.mult)
        else:
            nc.scalar.activation(dst, src, ACT.Identity, scale=1.0 / WSV)

    # ============ phase A: enc matmuls (PE) + groupnorm (DVE/ACT) ============
    pA = tc.tile_pool(name="psumA", bufs=2, space="PSUM")
    pGN = tc.tile_pool(name="psumGN", bufs=1, space="PSUM")
    with pA as pa_pool, pGN as pgn_pool:
        def emit_enc():
            # enc keys -> k_f8[j][0:S]
            for j in range(NPAIR):
                pek = pa_pool.tile([128, S], F32, tag="pvt", bufs=2,
                                   name=f"pek_{j}")
                wj = w_pair_view(wek_sb, j)
                for kp in range(NK):
                    nc.tensor.matmul(
                        pek[:], lhsT=wj[:, kp], rhs=enc_v[:, kp],
                        start=(kp == 0), stop=(kp == NK - 1), perf_mode=DR,
                    )
                nc.vector.tensor_scalar(
                    k_f8[j][:, 0:S], pek[:], 1.0,
                    bekc_sb[:, j: j + 1], op0=AOP.mult, op1=AOP.add,
                )

            # enc values transposed -> vt_dr[0] (+ ev bias via ones row)
            for st in range(2):
                pvt = pa_pool.tile([128, C], F32, tag="pvt", bufs=2,
                                   name=f"pvt_{st}")
                for cs in range(NK):
                    for kp in range(NK):
                        nc.tensor.matmul(
                            pvt[:, 256 * cs: 256 * (cs + 1)],
                            lhsT=enc_v[:, kp, :, 128 * st: 128 * (st + 1)],
                            rhs=wev_v[:, kp, :, 256 * cs: 256 * (cs + 1)],
                            start=(kp == 0 and cs != 1), stop=False,
                            perf_mode=DR, skip_group_check=True,
                        )
                    nc.tensor.matmul(
                        pvt[:, 256 * cs: 256 * (cs + 1)], lhsT=ones_r[0:1, :],
                        rhs=bev_sb[0:1, 256 * cs: 256 * (cs + 1)],
                        start=False, stop=True, skip_group_check=True,
                    )
                nc.vector.tensor_scalar(
                    vt4[0][:, :, st, :],
                    pvt[:].rearrange("p (h c) -> p h c", c=CH),
                    1.0 / WSV, None, op0=AOP.mult,
                )

        # ---- groupnorm stats: DVE bn_stats for tiles 0-2, ACT Copy/Square
        # accumulators for tiles 3-5 (both engines idle pre-stats, and the
        # chain gates the loop start) ----
        # bn_stats gives (n, mean, n*var) for even/odd elements per 512-group:
        # sum x = 256 * sum(means); sum x^2 = sum(M2) + 256 * sum(means^2)
        _prio_save = tc.cur_priority
        tc.cur_priority = 0
        NB = NP         # all tiles on the bn_stats path (DVE)
        bn6 = sqpool.tile([128, NB * 2 * 6], F32, tag="bn6")
        bn6v = bn6[:].rearrange("p (ct g s) -> p ct g s", ct=NB, g=2)
        for ct in range(NB):
            for g in range(2):
                nc.vector.bn_stats(
                    bn6v[:, ct, g], x_bf[ct][:, 512 * g: 512 * (g + 1)],
                )
        # the whole conversions/Newton chain stays on DVE: spreading it over
        # Pool creates a Pool.SEQ head-of-line stall (static in-order dispatch
        # interlocks with the DVE/ACT orders and blocks the shuffle legs)
        m4 = bn6[:].rearrange("p (ct g eo three) -> p ct g eo three",
                              ct=NB, g=2, eo=2)[:, :, :, :, 1]  # means
        v4 = bn6[:].rearrange("p (ct g eo three) -> p ct g eo three",
                              ct=NB, g=2, eo=2)[:, :, :, :, 2]  # n*var
        msum = sqpool.tile([128, NB * 4], F32, tag="msum")
        m2s = msum[:].rearrange("p (ct f) -> p ct f", f=4)
        s12b = s12_sb[:, 0: 2 * NB].rearrange("p (ct two) -> p ct two", two=2)
        # f0: mean_e0+mean_o0+mean_e1+mean_o1 ; f1: sum m^2 ; f2: sum M2
        nc.vector.tensor_tensor(m2s[:, :, 0:1], m4[:, :, 0, 0:1], m4[:, :, 0, 1:2],
                                op=AOP.add)
        nc.vector.tensor_tensor(m2s[:, :, 1:2], m4[:, :, 1, 0:1], m4[:, :, 1, 1:2],
                                op=AOP.add)
        nc.vector.tensor_tensor(m2s[:, :, 0:1], m2s[:, :, 0:1], m2s[:, :, 1:2],
                                op=AOP.add)
        nc.vector.tensor_scalar_mul(s12b[:, :, 0:1], m2s[:, :, 0:1], 256.0)
        sq4 = sqpool.tile([128, NB * 4], F32, tag="sq4")
        sq4v = sq4[:].rearrange("p (ct g eo) -> p ct g eo", ct=NB, g=2)
        nc.vector.tensor_tensor(sq4v[:], m4[:], m4[:], op=AOP.mult)
        nc.vector.tensor_tensor(sq4v[:, :, 0], sq4v[:, :, 0], sq4v[:, :, 1],
                                op=AOP.add)
        nc.vector.tensor_tensor(sq4v[:, :, 0, 0:1], sq4v[:, :, 0, 0:1],
                                sq4v[:, :, 0, 1:2], op=AOP.add)
        vsum = sqpool.tile([128, NB * 2], F32, tag="vsum")
        vs = vsum[:].rearrange("p (ct eo) -> p ct eo", eo=2)
        nc.vector.tensor_tensor(vs[:], v4[:, :, 0], v4[:, :, 1], op=AOP.add)
        nc.vector.tensor_tensor(vs[:, :, 0:1], vs[:, :, 0:1], vs[:, :, 1:2],
                                op=AOP.add)
        nc.vector.scalar_tensor_tensor(
            s12b[:, :, 1:2],
            in0=sq4v[:, :, 0, 0:1], scalar=256.0, in1=vs[:, :, 0:1],
            op0=AOP.mult, op1=AOP.add,
        )
        pst = pgn_pool.tile([32, 2], F32, tag="pst")
        for ct in range(NP):
            nc.tensor.matmul(
                pst[:], lhsT=ind_sb[:, NG * ct: NG * (ct + 1)],
                rhs=s12_sb[:, 2 * ct: 2 * ct + 2],
                start=(ct == 0), stop=(ct == NP - 1),
            )
        n_per_group = (C // NG) * T
        gm = spool.tile([32, 1], F32, tag="gm")
        gm2 = spool.tile([32, 1], F32, tag="gm2")
        var_t = spool.tile([32, 1], F32, tag="var")
        ab32 = spool.tile([32, 2], F32, tag="ab32")
        nc.vector.tensor_scalar_mul(gm[:], pst[:, 0:1], 1.0 / n_per_group)
        nc.vector.tensor_tensor(gm2[:], gm[:], gm[:], op=AOP.mult)
        nc.vector.scalar_tensor_tensor(
            var_t[:], in0=pst[:, 1:2], scalar=1.0 / n_per_group, in1=gm2[:],
            op0=AOP.mult, op1=AOP.subtract,
        )
        v_t = spool.tile([32, 1], F32, tag="veps")
        nc.vector.tensor_scalar_add(v_t[:], var_t[:], float(EPS))
        y0i = spool.tile([32, 1], mybir.dt.int32, tag="y0i")
        nc.vector.tensor_scalar(
            y0i[:], v_t[:].bitcast(mybir.dt.int32), 1, None,
            op0=AOP.arith_shift_right,
        )
        nc.vector.tensor_scalar(
            y0i[:], y0i[:], -1, 0x5F3759DF, op0=AOP.mult, op1=AOP.add,
        )
        y = y0i[:].bitcast(F32)
        h_t = spool.tile([32, 1], F32, tag="half_v")
        nc.vector.tensor_scalar_mul(h_t[:], v_t[:], 0.5)
        yy = spool.tile([32, 1], F32, tag="yy")
        r_t = spool.tile([32, 1], F32, tag="rt")
        for it in range(3):
            nc.vector.tensor_tensor(yy[:], y, y, op=AOP.mult)
            nc.vector.tensor_tensor(r_t[:], h_t[:], yy[:], op=AOP.mult)
            nc.vector.tensor_scalar(
                r_t[:], r_t[:], -1.0, 1.5, op0=AOP.mult, op1=AOP.add,
            )
            dst = ab32[:, 0:1] if it == 2 else y
            nc.vector.tensor_tensor(dst, y, r_t[:], op=AOP.mult)
        nc.vector.scalar_tensor_tensor(
            ab32[:, 1:2], in0=gm[:], scalar=-1.0, in1=ab32[:, 0:1],
            op0=AOP.mult, op1=AOP.mult,
        )
        pab = pgn_pool.tile([128, 2 * NP], F32, tag="pab")
        for ct in range(NP):
            nc.tensor.matmul(
                pab[:, 2 * ct: 2 * ct + 2],
                lhsT=indT_sb[:, 128 * ct: 128 * (ct + 1)], rhs=ab32[:],
                start=True, stop=True, skip_group_check=True,
            )
        pab_sb = spool.tile([128, 2 * NP], F32, tag="pabs")
        nc.vector.tensor_copy(pab_sb[:], pab[:])
        pab3 = pab_sb[:].rearrange("p (ct two) -> p ct two", two=2)
        ab3 = ab_sb[:].rearrange("p (ct two) -> p ct two", two=2)
        gn3 = gnw_sb[:].rearrange("p (ct one) -> p ct one", one=1)
        gb3 = gnb_sb[:].rearrange("p (ct one) -> p ct one", one=1)
        nc.vector.tensor_tensor(ab3[:, :, 0:1], pab3[:, :, 0:1], gn3, op=AOP.mult)
        nc.vector.tensor_tensor(ab3[:, :, 1:2], pab3[:, :, 1:2], gn3, op=AOP.mult)
        nc.vector.tensor_tensor(ab3[:, :, 1:2], ab3[:, :, 1:2], gb3, op=AOP.add)
        for ct in range(NP):
            dst = xn_v[ct // 2][:, ct % 2, :]
            if ct % 3 == 0:
                nc.vector.tensor_scalar(
                    dst, x_bf[ct][:],
                    ab_sb[:, 2 * ct: 2 * ct + 1], ab_sb[:, 2 * ct + 1: 2 * ct + 2],
                    op0=AOP.mult, op1=AOP.add,
                )
            elif ct % 3 == 1:
                nc.scalar.activation(
                    dst, x_bf[ct][:],
                    ACT.Identity, bias=ab_sb[:, 2 * ct + 1: 2 * ct + 2],
                    scale=ab_sb[:, 2 * ct: 2 * ct + 1],
                )
            else:
                nc.gpsimd.tensor_scalar(
                    dst, x_bf[ct][:],
                    ab_sb[:, 2 * ct: 2 * ct + 1], ab_sb[:, 2 * ct + 1: 2 * ct + 2],
                    op0=AOP.mult, op1=AOP.add,
                )
        tc.cur_priority = _prio_save

        # pair-0 AND pair-1 q/k production inside phase A (1 slot,
        # serialized); enc keys first so the k shuffles read fully-written
        # k_f8 tiles. Drains favor ACT (it idles through early phase A).
        pq = pa_pool.tile([128, T], F32, tag="pp0", bufs=1, name="pp0_q")
        emit_prod(0, "q", pq)
        emit_enc()
        pq = pa_pool.tile([128, T], F32, tag="pp0", bufs=1, name="pp0_k")
        emit_prod(0, "k", pq)
        for which in ("q", "k"):
            pq = pa_pool.tile([128, T], F32, tag="pp0", bufs=1,
                              name=f"pp1_{which}")
            emit_prod(1, which, pq)
        # all 8 self-vt tiles in phase A (the loop's pS rotation can't absorb
        # them; drains alternate DVE/ACT)
        for tt in range(8):
            pvt = pa_pool.tile([128, C], F32, tag="pvt", bufs=2,
                               name=f"pvs_{tt}")
            emit_vt_tt(tt, pvt)

    sqpool.release()
    early.release()

    # ==== pair loop: t-half-MAJOR (all pairs th=0, then th=1; 120 steps) ====
    # PSUM: pS 3x[128,1024] scores (both heads; productions and th0-proj ride
    # the same rotation) + pPA 1x[128,512] a^T accum + pPZ 1 bank (Z) = 8.
    pS = tc.alloc_tile_pool(name="psumS", bufs=3, space="PSUM")
    pPA = tc.alloc_tile_pool(name="psumPA", bufs=1, space="PSUM")
    pPZ = tc.alloc_tile_pool(name="psumPZ", bufs=1, space="PSUM")

    halves = [(j, 0) for j in range(NPAIR)] + [(j, 1) for j in range(NPAIR)]
    pa_t = {}
    E_tiles = {}
    exp_i = 0
    exp_acc = float(os.environ.get("K_EXPOFF", "0.0"))
    pz_t = pPZ.tile([128, 16], F32, tag="pz")

    def emit_scores_exp(j, th, st):
        # both heads' scores land in one [128, 1024] tile -> ONE exp op
        # (amortizes the per-op PSUM/SBUF access latency)
        nonlocal exp_i, exp_acc
        ps = pS.tile([128, 1024], F32, tag="ps", name=f"ps_{j}_{th}_{st}")
        for h in range(2):
            nc.tensor.matmul(
                ps[:, 512 * h: 512 * (h + 1)],
                lhsT=kd_v[j][:, h, :, 128 * st: 128 * (st + 1)],
                rhs=qd_v[j][:, h, :, 512 * th: 512 * (th + 1)],
                start=True, stop=True, perf_mode=DR, skip_group_check=True,
            )
        sp, par = st // 2, st % 2
        if par == 0:
            E_tiles[(j, th, sp)] = epool.tile(
                [128, 2 * 1024], FP8, tag="E", name=f"E_{j}_{th}_{sp}")
        E_t = E_tiles[(j, th, sp)]
        slot = E_t[:].rearrange("p (i n) -> p i n", i=2)[:, par, :]
        exp_i += 1
        _pat = os.environ.get("K_EXPPAT", "")
        if _pat:
            use_act = _pat[st % len(_pat)] == "a"
        else:
            exp_acc += EXP_DVE_FRAC
            use_act = exp_acc < 1.0
        if use_act:
            nc.scalar.activation(slot, ps[:], ACT.Exp, scale=1.0 / (WS * WS))
        else:
            if not _pat:
                exp_acc -= 1.0
            nc.vector.tensor_scalar(
                slot.bitcast(I8), ps[:],
                EXP8_A / (WS * WS), EXP8_B, op0=AOP.mult, op1=AOP.add,
            )

    def emit_pv(j, th, sp, hn):
        # E stationary (LdWeights free), vt moving: a^T[t-chunk, ch] chunks
        # accumulate over st-pairs; Z via 1-col matmuls vs fp8 ones.
        zoff = 8 * (hn % 2)
        E_t = E_tiles[(j, th, sp)] if sp < NSP - 1 else \
            E_tiles.pop((j, th, sp))
        ev = E_t[:].rearrange("p (i h n) -> p i h n", i=2, h=2)
        for h in range(2):
            for c in range(NCH):
                lhs = ev[:, :, h, 128 * c: 128 * (c + 1)]
                nc.tensor.matmul(
                    pa_t[(j, th)][:, 128 * c + 64 * h: 128 * c + 64 * h + 64],
                    lhsT=lhs, rhs=vt4[sp][:, 2 * j + h],
                    start=(sp == 0), stop=(sp == NSP - 1),
                    perf_mode=DR, skip_group_check=True,
                )
                nc.tensor.matmul(
                    pz_t[:, zoff + 2 * c + h: zoff + 2 * c + h + 1],
                    lhsT=lhs, rhs=ones_f8[:].unsqueeze(-1),
                    start=(sp == 0), stop=(sp == NSP - 1),
                    perf_mode=DR, skip_group_check=True,
                )

    def emit_normalize(j, th, hn, last=False):
        # zinv = 1/Z [128, 8]; a^T = pa * zinv (stride-0 broadcast along ch)
        # in bf16; then 4 DMA xbar transposes into a_sb[j] (c-major layout).
        # For the LAST half the DMA route's ~3.5us latency would idle the PE
        # (p-state drop) right before the tail proj -- use PE transposes.
        zoff = 8 * (hn % 2)
        _po = int(os.environ.get("K_NRMPRIO", "0"))
        zi = zpool.tile([128, 8], F32, tag="zi", name=f"zi_{j}_{th}")
        aT = zpool.tile([128, 512], BF16, tag="aT", name=f"aT_{j}_{th}")
        pa = pa_t.pop((j, th))
        zb = zi[:].rearrange("p (c h) -> p c h", h=2).unsqueeze(-1) \
            .broadcast_to([128, NCH, 2, CH])
        with tc.high_priority(offset=_po) if _po else _nullcm():
            nc.vector.reciprocal(zi[:], pz_t[:, zoff: zoff + 8])
            nc.vector.tensor_tensor(
                aT[:].rearrange("p (c h i) -> p c h i", c=NCH, h=2),
                pa[:].rearrange("p (c h i) -> p c h i", c=NCH, h=2),
                zb, op=AOP.mult,
            )
        if not last:
            for c in range(NCH):
                nc.sync.dma_start_transpose(
                    a_sb[j][:, 512 * th + 128 * c: 512 * th + 128 * (c + 1)],
                    aT[:, 128 * c: 128 * (c + 1)],
                )
            return
        for r in range(2):
            ptr = (pPA.tile([128, 256], BF16, tag="pa", name="ptr_0") if r == 0
                   else pPZ.tile([128, 256], BF16, tag="pz", name="ptr_1"))
            for c in (2 * r, 2 * r + 1):
                nc.tensor.transpose(
                    ptr[:, 128 * (c % 2): 128 * (c % 2) + 128],
                    aT[:, 128 * c: 128 * (c + 1)], idn_sb[:],
                )
            nc.vector.tensor_copy(
                a_sb[j][:, 512 * th + 256 * r: 512 * th + 256 * (r + 1)],
                ptr[:],
            )

    def emit_proj(ot, half, pool, tag, quarter=None):
        # quarter=None: full 512-wide chain; quarter=0/1: 256-wide half-chain
        # (smaller pS slot-hold when riding the loop's scores rotation)
        w, off = (512, 512 * half) if quarter is None else             (256, 512 * half + 256 * quarter)
        ph = pool.tile([128, w], F32, tag=tag, name=f"ph_{ot}_{half}_{quarter}")
        for j in range(NPAIR):
            nc.tensor.matmul(
                ph[:], lhsT=wpt_v[:, j, ot],
                rhs=a_sb[j][:, off: off + w],
                start=(j == 0), stop=False,
                skip_group_check=True,
            )
        # residual: x folded into the accumulator via identity matmul
        nc.tensor.matmul(
            ph[:], lhsT=idn_sb[:],
            rhs=x_bf[ot][:, off: off + w],
            start=False, stop=True, skip_group_check=True,
        )
        o_t = opool.tile([128, w], F32, tag="out", name=f"o_{ot}_{half}_{quarter}")
        nc.scalar.activation(o_t[:], ph[:], ACT.Identity)
        nc.sync.dma_start(
            out_ap[128 * ot: 128 * (ot + 1), off: off + w],
            o_t[:],
        )

    for hn, (j, th) in enumerate(halves):
        for st in range(NS):
            if st == 0:
                pa_t[(j, th)] = pPA.tile([128, 512], F32, tag="pa",
                                         name=f"pa_{j}_{th}")
            emit_scores_exp(j, th, st)
            # PV schedule: PV(cur, sp) at st = 2*sp+3 for sp 0..3; PV(prev, 4)
            # at st=1; normalize(prev) at st=2.
            _pvl = int(__import__("os").environ.get("K_PVLAG", "3"))
            _pvt = int(__import__("os").environ.get("K_PVTAIL", "0"))
            if st == max(0, _pvl - 2 - _pvt) and hn >= 1:
                emit_pv(*halves[hn - 1], 4, hn - 1)
            elif st == max(1, _pvl - 1 - _pvt) and hn >= 1:
                emit_normalize(*halves[hn - 1], hn - 1)
            elif st >= _pvl and (st - _pvl) % 2 == 0:
                emit_pv(j, th, (st - _pvl) // 2, hn)
            # productions (th=0 halves), two-half lead so the shuffle DMA
            # latency (~3us: drain + SWDGE/HWDGE gen + transfer + sem) is
            # fully hidden: pairs 0-1 from phase A; hn j makes pair j+2
            pj = None
            _pp = int(__import__("os").environ.get("K_PRST", "4")) \
                + int(__import__("os").environ.get("K_PRSTAG", "0")) * (j % 2)
            if th == 0 and j <= 3 and st in (_pp, _pp + 1):
                pj, which = j + 2, ("q" if st == _pp else "k")
            if pj is not None:
                pq = pS.tile([128, T], F32, tag="ps", name=f"pq_{pj}_{which}")
                emit_prod(pj, which, pq, eng=nc.scalar)
            # th=0 proj chains ride the pS rotation during th=1, one per half
            _pjs = int(__import__("os").environ.get("K_PJST", "4"))
            _pjg = int(__import__("os").environ.get("K_PJSTAG", "-2"))
            if 6 <= hn <= 11 and st == _pjs + _pjg * ((hn - 6) % 2):
                emit_proj(hn - 6, 0, pS, "ps")

    # ============ tail: last PV + normalize + th=1 proj ============
    emit_pv(NPAIR - 1, 1, 4, len(halves) - 1)
    emit_normalize(NPAIR - 1, 1, len(halves) - 1, last=True)
    for ot in range(NP):
        emit_proj(ot, 1, pS, "ps")

    pPZ.release()
    pPA.release()
    pS.release()


def _prep_host(inputs):
    """Host-side weight prep. Returns (shared, per_core)."""
    x = np.ascontiguousarray(inputs["x"], dtype=np.float32).reshape(B, C, T)
    enc = np.ascontiguousarray(inputs["encoder_out"], dtype=np.float32)
    qkv_w = np.asarray(inputs["qkv_w"], np.float32)
    qkv_b = np.asarray(inputs["qkv_b"], np.float32)
    enc_w = np.asarray(inputs["enc_w"], np.float32)
    enc_b = np.asarray(inputs["enc_b"], np.float32)
    proj_w = np.asarray(inputs["proj_w"], np.float32)
    proj_b = np.asarray(inputs["proj_b"], np.float32)
    gn_w = np.asarray(inputs["gn_w"], np.float32)
    gn_b = np.asarray(inputs["gn_b"], np.float32)

    qkv_r = qkv_w.reshape(NH, 3 * CH, C)
    q_w = (qkv_r[:, :CH] * SCALE).reshape(NH * CH, C)
    k_w = (qkv_r[:, CH:2 * CH] * SCALE).reshape(NH * CH, C)
    v_w = qkv_r[:, 2 * CH:].reshape(NH * CH, C)
    qb = qkv_b.reshape(NH, 3 * CH)
    q_b = (qb[:, :CH] * SCALE).reshape(-1)
    k_b = (qb[:, CH:2 * CH] * SCALE).reshape(-1)
    v_b = qb[:, 2 * CH:].reshape(-1)
    enc_r = enc_w.reshape(NH, 2 * CH, C)
    ek_w = (enc_r[:, :CH] * SCALE).reshape(NH * CH, C)
    ev_w = enc_r[:, CH:].reshape(NH * CH, C)
    eb = enc_b.reshape(NH, 2 * CH)
    ek_b = (eb[:, :CH] * SCALE).reshape(-1)
    ev_b = eb[:, CH:].reshape(-1)
    hb = proj_w @ v_b + proj_b
    assert not np.any(hb), "nonzero v/proj bias not supported by v3 kernel"

    def dr_lhsT(w, scale):
        # w [out 768, in 768] -> [p, j, kp, i, m] = w[j*128+m, (2*kp+i)*128+p]
        a = (w * scale).reshape(6, 128, NK, 2, 128)   # [j, m, kp, i, p]
        a = a.transpose(4, 0, 2, 3, 1)                # [p, j, kp, i, m]
        return np.ascontiguousarray(a.reshape(128, -1)).astype(NPF8)

    def dr_rhs(w, scale):
        # w [out 768, in 768] -> [p, kp, i, n] = w[n, (2kp+i)*128+p]
        a = (w * scale).reshape(768, NK, 2, 128)      # [n, kp, i, p]
        a = a.transpose(3, 1, 2, 0)                   # [p, kp, i, n]
        return np.ascontiguousarray(a.reshape(128, -1)).astype(NPF8)

    # proj lhsT bf16: [p, j, ot, m] = proj_w[128*ot+m, 128*j+p]
    wpt = proj_w.reshape(NP, 128, NPAIR, 128)        # [ot, m, j, p]
    wpt = wpt.transpose(3, 2, 0, 1)                  # [p, j, ot, m]
    wpt = np.ascontiguousarray(wpt.reshape(128, -1)).astype(NPBF)

    ind = np.zeros((C, NG), np.float32)
    ind[np.arange(C), np.arange(C) // (C // NG)] = 1.0

    def colmaj(v):
        return np.ascontiguousarray(v.reshape(6, 128).T).astype(np.float32)

    shared = {
        "wq": dr_lhsT(q_w, WS), "wk": dr_lhsT(k_w, WS),
        "wek": dr_lhsT(ek_w, WS),
        "wv": dr_rhs(v_w, WSV), "wev": dr_rhs(ev_w, WSV),
        "wpt": wpt,
        "bev": (ev_b * WSV).reshape(1, C).astype(NPBF),
        "bqc": colmaj(q_b * WS), "bkc": colmaj(k_b * WS),
        "bekc": colmaj(ek_b * WS),
        "gnw": colmaj(gn_w), "gnb": colmaj(gn_b),
        "ind": ind, "indT": np.ascontiguousarray(ind.T),
        "idn": np.eye(128, dtype=NPBF),
    }
    per_core = []
    for b in range(B):
        e = enc[b].reshape(NK, 2, 128, S).transpose(2, 0, 1, 3).reshape(128, -1)
        per_core.append({
            "xbf": np.ascontiguousarray(x[b]).astype(NPBF),
            "enc": np.ascontiguousarray(e).astype(NPF8),
        })
    return shared, per_core


def _declare(nc):
    def di(name, shape, dt):
        return nc.dram_tensor(name, shape, dt, kind="ExternalInput").ap()

    ins = {
        "xbf": di("xbf", [C, T], BF16),
        "enc": di("enc", [128, NK * 2 * S], FP8),
        "wq": di("wq", [128, NPAIR * NK * 2 * 128], FP8),
        "wk": di("wk", [128, NPAIR * NK * 2 * 128], FP8),
        "wek": di("wek", [128, NPAIR * NK * 2 * 128], FP8),
        "wv": di("wv", [128, NK * 2 * C], FP8),
        "wev": di("wev", [128, NK * 2 * C], FP8),
        "wpt": di("wpt", [128, NPAIR * NP * 128], BF16),
        "bev": di("bev", [1, C], BF16),
        "bqc": di("bqc", [128, NPAIR], F32),
        "bkc": di("bkc", [128, NPAIR], F32),
        "bekc": di("bekc", [128, NPAIR], F32),
        "gnw": di("gnw", [128, NP], F32), "gnb": di("gnb", [128, NP], F32),
        "ind": di("ind", [C, NG], F32), "indT": di("indT", [NG, C], F32),
        "idn": di("idn", [128, 128], BF16),
    }
    out = nc.dram_tensor("out", [C, T], F32, kind="ExternalOutput").ap()
    return ins, out


def build_nc():
    nc = bacc.Bacc("TRN2", target_bir_lowering=False, debug=False)
    ins, out = _declare(nc)
    with tile.TileContext(nc) as tc:
        with ExitStack() as stack:
            tc._ctx = stack
            _emit(tc, ins, out)
    nc.compile()
    return nc


_NC_CACHE = {}


def run(inputs, trace=False):
    shared, per_core = _prep_host(inputs)
    if "nc" not in _NC_CACHE:
        _NC_CACHE["nc"] = build_nc()
    nc = _NC_CACHE["nc"]
    in_maps = [dict(shared, **pc) for pc in per_core]
    last_err = None
    for attempt in range(3):
        try:
            res = run_bass_kernel_spmd(nc, in_maps, list(range(B)), trace=trace)
            break
        except Exception as e:
            last_err = e
            if attempt == 2:
                raise
            import time
            time.sleep(15)
    outs = np.stack([r["out"] for r in res.results])  # [B, C, T]
    return outs.reshape(B, C, HH, WW).astype(np.float32), res


def kernel(**inputs):
    out, _ = run(inputs, trace=False)
    return out
